# revision 1
# baseline (speedup 1.0000x reference)
"""CTC loss (mean, zero_infinity, target-length normalized) on 8 Trainium2 cores.

Sharding (per the hint): data-parallel over batch N — each core takes 8 of the
64 sequences and computes their per-sequence normalized NLL fully on device;
the host mean over the 64 values is the unshard step.

Device algorithm per core (exact log-space, numerically identical in
structure to the reference):
  Phase 1 (memory-bound): stream preds (T,8,C); ACT Exp+accumulate produces
    the per-(t,n) softmax denominator; a GPSIMD indirect_copy gathers the
    128 label columns per sequence; ACT Identity with per-partition bias
    forms lp(t,n,s) = pred_gathered - ln(sumexp); results stream to HBM in
    (t, n, s) layout (plus the blank column).
  Phase 2: per-t log-space CTC forward recursion on (8, L) state tiles:
    alpha'(s) = logaddexp3(alpha(s), alpha(s-1), alpha(s-2)+logskip(s)) + lp_t(s)
    with logaddexp3 = m + ln(e^(a-m)+e^(b-m)+e^(c-m)), m the 3-way max.
  Final: nll = -(logaddexp of the two end lanes) / len, on device.
"""
import sys
sys.path.insert(0, "/opt/trn_rl_repo")

import numpy as np

import concourse.bass as bass
import concourse.bacc as bacc
import concourse.tile as tile
from concourse import mybir
from concourse.bass_utils import run_bass_kernel_spmd

T_FULL, N_FULL, C, S = 1024, 64, 512, 128
L = 2 * S + 1
NCORES = 8
NL = N_FULL // NCORES
NEG = -1.0e9
F32 = mybir.dt.float32
U16 = mybir.dt.uint16
AF = mybir.ActivationFunctionType
OP = mybir.AluOpType
TBLK = 16            # DP t-block size for streaming lp slices

_COMPILED = {}


def build_program(T):
    nc = bacc.Bacc("TRN2", target_bir_lowering=False, debug=False)

    preds = nc.dram_tensor("preds", [T, NL, C], F32, kind="ExternalInput")
    gidx = nc.dram_tensor("gidx", [128, 64], U16, kind="ExternalInput")
    lskip = nc.dram_tensor("lskip", [NL, L], F32, kind="ExternalInput")
    e1m = nc.dram_tensor("e1m", [NL, L], F32, kind="ExternalInput")
    e2m = nc.dram_tensor("e2m", [NL, L], F32, kind="ExternalInput")
    invlen = nc.dram_tensor("invlen", [NL, 1], F32, kind="ExternalInput")
    nll = nc.dram_tensor("nll", [NL, 1], F32, kind="ExternalOutput")

    # lp lattice in HBM, (t, n, s) with s = 0 blank, 1..128 labels
    lpd = nc.dram_tensor("lpd", [T, NL, 132], F32)

    n_ttiles = T // 128

    with tile.TileContext(nc) as tc:
        with (
            tc.tile_pool(name="p1", bufs=2) as p1,
            tc.tile_pool(name="p1s", bufs=2) as p1s,
            tc.tile_pool(name="const", bufs=1) as constp,
            tc.tile_pool(name="dp", bufs=1) as dpp,
            tc.tile_pool(name="lps", bufs=3) as lpsp,
        ):
            # ---------------- constants ----------------
            t_gidx = constp.tile([128, 64], U16)
            nc.gpsimd.dma_start(t_gidx[:], gidx[:])
            t_lsk = constp.tile([NL, L], F32)
            nc.sync.dma_start(t_lsk[:], lskip[:])
            t_e1 = constp.tile([NL, L], F32)
            nc.sync.dma_start(t_e1[:], e1m[:])
            t_e2 = constp.tile([NL, L], F32)
            nc.sync.dma_start(t_e2[:], e2m[:])
            t_invl = constp.tile([NL, 1], F32)
            nc.sync.dma_start(t_invl[:], invlen[:])

            # ---------------- phase 1 ----------------
            for tt in range(n_ttiles):
                ts0 = tt * 128
                tp = p1.tile([128, NL * C], F32, tag="tp")
                nc.gpsimd.dma_start(
                    tp[:], preds[ts0:ts0 + 128].rearrange("t n c -> t (n c)"))
                es = p1s.tile([128, C], F32, tag="es")
                sm = p1s.tile([128, NL], F32, tag="sm")
                for n in range(NL):
                    nc.scalar.activation(es[:], tp[:, n * C:(n + 1) * C],
                                         AF.Exp, accum_out=sm[:, n:n + 1])
                lns = p1s.tile([128, NL], F32, tag="lns")
                nc.scalar.activation(lns[:], sm[:], AF.Ln)
                nb = p1s.tile([128, NL], F32, tag="nb")
                nc.vector.tensor_scalar(nb[:], lns[:], -1.0, 0.0,
                                        OP.mult, OP.add)
                g = p1s.tile([128, NL * S], F32, tag="g")
                nc.gpsimd.indirect_copy(g[:], tp[:], t_gidx[:], True)
                # lp tile (128, NL*132): [blank, 128 labels, pad, pad, pad]
                lp = p1s.tile([128, NL * 132], F32, tag="lp")
                lp3 = lp[:].rearrange("t (n k) -> t n k", k=132)
                nc.vector.memset(lp3[:, :, 129:132], 0.0)
                for n in range(NL):
                    nc.scalar.activation(
                        lp3[:, n, 1:129], g[:, n * S:(n + 1) * S],
                        AF.Identity, bias=nb[:, n:n + 1], scale=1.0)
                # blank column
                bl = p1s.tile([128, NL], F32, tag="bl")
                nc.vector.tensor_copy(
                    bl[:], tp[:].rearrange("t (n c) -> t n c", c=C)[:, :, 0])
                nc.vector.tensor_sub(bl[:], bl[:], lns[:])
                nc.vector.tensor_copy(lp3[:, :, 0], bl[:])
                nc.sync.dma_start(
                    lpd[ts0:ts0 + 128].rearrange("t n k -> t (n k)"), lp[:])

            # ---------------- phase 2: per-t log-space DP ----------------
            # state layout: cols 0,1 guard NEG; col 2+s = alpha(s)
            Wd = L + 2
            stA = dpp.tile([NL, Wd], F32)
            stB = dpp.tile([NL, Wd], F32)
            t1 = dpp.tile([NL, L], F32)
            t2 = dpp.tile([NL, L], F32)
            m = dpp.tile([NL, L], F32)
            e1 = dpp.tile([NL, L], F32)
            e2 = dpp.tile([NL, L], F32)
            e3 = dpp.tile([NL, L], F32)
            lpt = dpp.tile([NL, L], F32)
            nc.vector.memset(stA[:], NEG)
            nc.vector.memset(stB[:], NEG)
            state = [stA, stB]

            for blk in range(T // TBLK):
                lpb = lpsp.tile([NL, TBLK * 132], F32, tag="lpb")
                nc.sync.dma_start(
                    lpb[:].rearrange("n (t k) -> n t k", k=132),
                    lpd[blk * TBLK:(blk + 1) * TBLK].rearrange("t n k -> n t k"))
                for j in range(TBLK):
                    t = blk * TBLK + j
                    X = state[(t + 1) % 2]
                    Y = state[t % 2]
                    lps = lpb[:, j * 132:j * 132 + 132]
                    # build lp_t over lanes: even lanes = blank, odd = labels
                    # lpt[s even] = lps[0]... do via two strided copies
                    if t == 0:
                        # alpha(0, 0) = lp_blank(0); alpha(0, 1) = lp_label0(0)
                        nc.vector.tensor_copy(Y[:, 2:3], lps[:, 0:1])
                        nc.vector.tensor_copy(Y[:, 3:4], lps[:, 1:2])
                        continue
                    # m = max(alpha, alpha_sh1, alpha_sh2 + lskip)
                    nc.vector.tensor_max(t1[:], X[:, 2:2 + L], X[:, 1:1 + L])
                    nc.vector.tensor_add(t2[:], X[:, 0:L], t_lsk[:])
                    nc.vector.tensor_max(m[:], t1[:], t2[:])
                    # exps
                    nc.vector.tensor_sub(e1[:], X[:, 2:2 + L], m[:])
                    nc.scalar.activation(e1[:], e1[:], AF.Exp)
                    nc.vector.tensor_sub(e2[:], X[:, 1:1 + L], m[:])
                    nc.scalar.activation(e2[:], e2[:], AF.Exp)
                    nc.vector.tensor_sub(e3[:], t2[:], m[:])
                    nc.scalar.activation(e3[:], e3[:], AF.Exp)
                    nc.vector.tensor_add(e1[:], e1[:], e2[:])
                    nc.vector.tensor_add(e1[:], e1[:], e3[:])
                    nc.scalar.activation(e1[:], e1[:], AF.Ln)
                    nc.vector.tensor_add(m[:], m[:], e1[:])
                    # + lp_t : even lanes get blank col, odd lanes label cols
                    nc.vector.tensor_scalar_add(
                        Y[:, 2:2 + L:2], m[:, 0:L:2], lps[:, 0:1])
                    nc.vector.tensor_add(
                        Y[:, 3:2 + L:2], m[:, 1:L:2], lps[:, 1:129])
                X = state[(T - 1) % 2]

            # ---------------- final ----------------
            Xf = state[(T - 1) % 2]
            R1 = dpp.tile([NL, 1], F32)
            R2 = dpp.tile([NL, 1], F32)
            M1 = dpp.tile([NL, 1], F32)
            M2 = dpp.tile([NL, 1], F32)
            # masked extraction of the two end lanes (mask elsewhere ~0,
            # add NEG*(1-mask) is avoided by using max-reduce on alpha+mask)
            nc.vector.tensor_add(t1[:], Xf[:, 2:2 + L], t_e1[:])
            nc.vector.tensor_reduce(R1[:], t1[:], mybir.AxisListType.X, OP.max)
            nc.vector.tensor_add(t1[:], Xf[:, 2:2 + L], t_e2[:])
            nc.vector.tensor_reduce(R2[:], t1[:], mybir.AxisListType.X, OP.max)
            nc.vector.tensor_max(M1[:], R1[:], R2[:])
            nc.vector.tensor_tensor(M2[:], R1[:], R2[:], op=OP.min)
            nc.vector.tensor_sub(M2[:], M2[:], M1[:])
            nc.scalar.activation(M2[:], M2[:], AF.Exp)
            nc.vector.tensor_scalar_add(M2[:], M2[:], 1.0)
            nc.scalar.activation(M2[:], M2[:], AF.Ln)
            nc.vector.tensor_add(M1[:], M1[:], M2[:])
            nc.vector.tensor_scalar(M1[:], M1[:], -1.0, 0.0, OP.mult, OP.add)
            out = dpp.tile([NL, 1], F32)
            nc.vector.tensor_mul(out[:], M1[:], t_invl[:])
            nc.sync.dma_start(nll[:], out[:])

    nc.compile()
    return nc


def _host_prep(preds, labels, label_lengths, T):
    labels = np.asarray(labels).astype(np.int64)
    ll = np.asarray(label_lengths).astype(np.int64)
    in_maps = []
    for c in range(NCORES):
        ns = slice(c * NL, (c + 1) * NL)
        lab = labels[ns]
        lln = ll[ns]
        vals = np.zeros(1024, dtype=np.uint16)
        for n in range(NL):
            vals[n * S: (n + 1) * S] = (n * C + lab[n]).astype(np.uint16)
        wrap = np.zeros((16, 64), dtype=np.uint16)
        for j in range(1024):
            wrap[j % 16, j // 16] = vals[j]
        gidx = np.tile(wrap, (8, 1))
        lskip = np.full((NL, L), NEG, dtype=np.float32)
        for n in range(NL):
            for i in range(1, S):
                if lab[n, i] != lab[n, i - 1]:
                    lskip[n, 2 * i + 1] = 0.0
        # end-lane extraction masks: 0 at the end lane, NEG elsewhere
        e1 = np.full((NL, L), NEG, dtype=np.float32)
        e2 = np.full((NL, L), NEG, dtype=np.float32)
        for n in range(NL):
            e1[n, 2 * lln[n]] = 0.0
            e2[n, 2 * lln[n] - 1] = 0.0
        invlen = (1.0 / np.maximum(lln, 1)).astype(np.float32).reshape(NL, 1)
        in_maps.append({
            "preds": np.ascontiguousarray(preds[:, ns, :], dtype=np.float32),
            "gidx": gidx,
            "lskip": lskip,
            "e1m": e1,
            "e2m": e2,
            "invlen": invlen,
        })
    return in_maps


def run_device(preds, labels, label_lengths, T=T_FULL, trace=False):
    if T not in _COMPILED:
        _COMPILED[T] = build_program(T)
    nc = _COMPILED[T]
    in_maps = _host_prep(preds, labels, label_lengths, T)
    res = run_bass_kernel_spmd(nc, in_maps, list(range(NCORES)), trace=trace)
    nlls = np.concatenate([r["nll"].reshape(NL) for r in res.results])
    return nlls, res


def kernel(preds, labels, input_lengths, label_lengths):
    preds = np.asarray(preds)
    labels = np.asarray(labels)
    input_lengths = np.asarray(input_lengths)
    label_lengths = np.asarray(label_lengths)
    assert preds.shape == (T_FULL, N_FULL, C)
    assert int(input_lengths.min()) == T_FULL and int(input_lengths.max()) == T_FULL, \
        "kernel specialized for full-length inputs"
    nlls, _ = run_device(preds, labels, label_lengths)
    # zero_infinity: saturated/non-finite -> 0 (reference semantics)
    nlls = np.where(np.isfinite(nlls) & (np.abs(nlls) < 1e6), nlls, 0.0)
    return np.float32(np.mean(nlls))



# revision 3
# speedup vs baseline: 1.1760x; 1.1760x over previous
"""CTC loss (mean, zero_infinity, target-length normalized) on 8 Trainium2 cores.

Sharding: data-parallel over batch N - each core takes 8 of the 64 sequences,
computes their per-sequence normalized NLL on device; the host mean over the
64 values is the unshard step.

Device algorithm per core (linear/probability space, validated vs f64 on host):
  Phase 1: stream preds (T,8,C); ACT Exp+accumulate produces the per-(t,n)
    softmax denominator; a GPSIMD indirect gather builds a 260-wide
    interleaved lane vector per (t,n): even lanes = blank, odd lanes =
    labels, lanes beyond 2*label_len point at a -1e9 column; ACT Exp with
    per-partition bias (-ln sumexp + lnK) forms p~(t,n,s) = K*softmax prob
    (0 for masked lanes). Streams to HBM as (t, n, 260).
  Phase 2: per-t linear-space CTC forward recursion on (8, 257) state:
    alpha'(s) = (alpha(s) + alpha(s-1) + sk01(s)*alpha(s-2)) * p~(t,s)
    with a scalar renormalization by 1/max every R=8 steps (log accumulated
    on the side); the boost K = e^7 per step keeps the running max centered
    in f32 range. 4 DVE ops per step; renorm adds 2 ops every 8 steps and
    folds its scale into the next step's multiply (scalar_tensor_tensor).
  Final: nll = -(ln(sum of the two end lanes) + sum ln(renorm maxes)
                - T*lnK) / label_len, on device.
"""
import sys
sys.path.insert(0, "/opt/trn_rl_repo")

import numpy as np

import concourse.bass as bass
import concourse.bacc as bacc
import concourse.tile as tile
from concourse import mybir
from concourse.bass_utils import run_bass_kernel_spmd

T_FULL, N_FULL, C, S = 1024, 64, 512, 128
L = 2 * S + 1            # 257 lanes
W = 260                  # stored lane width (257 + 3 pad)
NCORES = 8
NL = N_FULL // NCORES
NEG = -1.0e9
LNK = 7.0                # per-step lattice boost ln K
REN = 8                  # renorm interval
F32 = mybir.dt.float32
U16 = mybir.dt.uint16
AF = mybir.ActivationFunctionType
OP = mybir.AluOpType
TBLK = 16                # lattice t-block size streamed into SBUF

_COMPILED = {}


def build_program(T):
    nc = bacc.Bacc("TRN2", target_bir_lowering=False, debug=False)

    preds = nc.dram_tensor("preds", [T, NL, C], F32, kind="ExternalInput")
    gidx = nc.dram_tensor("gidx", [128, W * NL // 16], U16, kind="ExternalInput")
    skodd = nc.dram_tensor("skodd", [NL, S], F32, kind="ExternalInput")
    em01 = nc.dram_tensor("em01", [NL, W], F32, kind="ExternalInput")
    ninvl = nc.dram_tensor("ninvl", [NL, 1], F32, kind="ExternalInput")
    nll = nc.dram_tensor("nll", [NL, 1], F32, kind="ExternalOutput")

    # p~ lattice in HBM, (t, n, s) interleaved lanes
    lpd = nc.dram_tensor("lpd", [T, NL, W], F32)

    n_ttiles = T // 128
    n_ren = (T - 2) // REN  # renorms at t = REN-1, 2*REN-1, ..., < T-1

    with tile.TileContext(nc) as tc:
        with (
            tc.tile_pool(name="p1", bufs=2) as p1,
            tc.tile_pool(name="p1s", bufs=2) as p1s,
            tc.tile_pool(name="const", bufs=1) as constp,
            tc.tile_pool(name="dp", bufs=1) as dpp,
            tc.tile_pool(name="lps", bufs=3) as lpsp,
        ):
            # ---------------- constants ----------------
            t_gidx = constp.tile([128, W * NL // 16], U16)
            nc.gpsimd.dma_start(t_gidx[:], gidx[:])
            t_sk = constp.tile([NL, S], F32)
            nc.sync.dma_start(t_sk[:], skodd[:])
            t_em = constp.tile([NL, W], F32)
            nc.sync.dma_start(t_em[:], em01[:])
            t_nil = constp.tile([NL, 1], F32)
            nc.sync.dma_start(t_nil[:], ninvl[:])

            # ---------------- phase 1 ----------------
            for tt in range(n_ttiles):
                ts0 = tt * 128
                tp = p1.tile([128, NL * C + 4], F32, tag="tp")
                nc.gpsimd.dma_start(
                    tp[:, :NL * C],
                    preds[ts0:ts0 + 128].rearrange("t n c -> t (n c)"))
                nc.vector.memset(tp[:, NL * C:], NEG)
                es = p1s.tile([128, C], F32, tag="es")
                sm = p1s.tile([128, NL], F32, tag="sm")
                for n in range(NL):
                    nc.scalar.activation(es[:], tp[:, n * C:(n + 1) * C],
                                         AF.Exp, accum_out=sm[:, n:n + 1])
                lns = p1s.tile([128, NL], F32, tag="lns")
                nc.scalar.activation(lns[:], sm[:], AF.Ln)
                # nb = -ln(sumexp) + lnK
                nb = p1s.tile([128, NL], F32, tag="nb")
                nc.vector.tensor_scalar(nb[:], lns[:], -1.0, LNK,
                                        OP.mult, OP.add)
                g = p1s.tile([128, NL * W], F32, tag="g")
                # ISA caps indirect-copy dst at 1024 elems/partition: chunk it
                for o in range(0, NL * W, 1024):
                    sz = min(1024, NL * W - o)
                    nc.gpsimd.indirect_copy(
                        g[:, o:o + sz], tp[:],
                        t_gidx[:, o // 16:(o + sz) // 16], True)
                lp = p1s.tile([128, NL * W], F32, tag="lp")
                for n in range(NL):
                    nc.scalar.activation(
                        lp[:, n * W:(n + 1) * W], g[:, n * W:(n + 1) * W],
                        AF.Exp, bias=nb[:, n:n + 1], scale=1.0)
                nc.sync.dma_start(
                    lpd[ts0:ts0 + 128].rearrange("t n k -> t (n k)"), lp[:])

            # ---------------- phase 2: per-t linear-space DP ----------------
            # state: col 0,1 guards (0); col 2+s = alpha(s), s in [0, 257)
            Wd = L + 3
            stA = dpp.tile([NL, Wd], F32)
            stB = dpp.tile([NL, Wd], F32)
            u = dpp.tile([NL, S], F32)
            mxb = dpp.tile([NL, n_ren + 1], F32)
            rcp = dpp.tile([NL, 1], F32)
            nc.vector.memset(stA[:], 0.0)
            nc.vector.memset(stB[:], 0.0)
            state = [stA, stB]

            for blk in range(T // TBLK):
                lpb = lpsp.tile([NL, TBLK * W], F32, tag="lpb")
                nc.sync.dma_start(
                    lpb[:].rearrange("n (t k) -> n t k", k=W),
                    lpd[blk * TBLK:(blk + 1) * TBLK].rearrange(
                        "t n k -> n t k"))
                for j in range(TBLK):
                    t = blk * TBLK + j
                    X = state[(t + 1) % 2]
                    Y = state[t % 2]
                    pt = lpb[:, j * W:j * W + L]
                    if t == 0:
                        # alpha(0,0) = p(0,blank); alpha(0,1) = p(0,lab0)
                        nc.vector.tensor_copy(Y[:, 2:4], pt[:, 0:2])
                        continue
                    # op1: u = alpha(s-2)|odd * sk01
                    nc.vector.tensor_mul(u[:], X[:, 1:2 + L - 2:2], t_sk[:])
                    # op2: Y = alpha(s) + alpha(s-1)
                    nc.vector.tensor_add(Y[:, 2:2 + L], X[:, 2:2 + L],
                                         X[:, 1:1 + L])
                    # op3: Y|odd += u
                    nc.vector.tensor_add(Y[:, 3:2 + L:2], Y[:, 3:2 + L:2],
                                         u[:])
                    # op4: Y *= p_t  (folding in 1/max after a renorm)
                    if t % REN == 0 and t >= REN:
                        nc.vector.scalar_tensor_tensor(
                            Y[:, 2:2 + L], Y[:, 2:2 + L], rcp[:, 0:1], pt,
                            OP.mult, OP.mult)
                    else:
                        nc.vector.tensor_mul(Y[:, 2:2 + L], Y[:, 2:2 + L], pt)
                    # renorm bookkeeping every REN steps
                    if t % REN == REN - 1 and t < T - 1:
                        jr = t // REN
                        nc.vector.tensor_reduce(
                            mxb[:, jr:jr + 1], Y[:, 2:2 + L],
                            mybir.AxisListType.X, OP.max)
                        nc.vector.reciprocal(rcp[:], mxb[:, jr:jr + 1])

            # ---------------- final ----------------
            Xf = state[(T - 1) % 2]
            tot = dpp.tile([NL, L], F32)
            r = dpp.tile([NL, 1], F32)
            lnm = dpp.tile([NL, n_ren + 1], F32)
            s1 = dpp.tile([NL, 1], F32)
            nc.vector.tensor_mul(tot[:], Xf[:, 2:2 + L], t_em[:, :L])
            nc.vector.tensor_reduce(r[:], tot[:], mybir.AxisListType.X, OP.add)
            nc.scalar.activation(r[:], r[:], AF.Ln)
            nc.scalar.activation(lnm[:, :n_ren], mxb[:, :n_ren], AF.Ln)
            nc.vector.tensor_reduce(s1[:], lnm[:, :n_ren],
                                    mybir.AxisListType.X, OP.add)
            nc.vector.tensor_add(r[:], r[:], s1[:])
            # nll = -(r - T*lnK)/len = (r - T*lnK) * ninvl
            nc.vector.tensor_scalar_add(r[:], r[:], -float(T) * LNK)
            out = dpp.tile([NL, 1], F32)
            nc.vector.tensor_mul(out[:], r[:], t_nil[:])
            nc.sync.dma_start(nll[:], out[:])

    nc.compile()
    return nc


def _host_prep(preds, labels, label_lengths, T):
    labels = np.asarray(labels).astype(np.int64)
    ll = np.asarray(label_lengths).astype(np.int64)
    NEGCOL = NL * C  # column memset to -1e9 in the tp tile
    in_maps = []
    for c in range(NCORES):
        ns = slice(c * NL, (c + 1) * NL)
        lab = labels[ns]
        lln = ll[ns]
        vals = np.zeros(NL * W, dtype=np.uint16)
        for n in range(NL):
            for s in range(W):
                j = n * W + s
                if s >= L:
                    vals[j] = NEGCOL
                elif s % 2 == 0:       # blank lane 2i, alive iff i <= ll
                    vals[j] = (n * C) if (s // 2) <= lln[n] else NEGCOL
                else:                  # label lane 2i+1, alive iff i < ll
                    i = (s - 1) // 2
                    vals[j] = (n * C + lab[n, i]) if i < lln[n] else NEGCOL
        wrap = np.zeros((16, NL * W // 16), dtype=np.uint16)
        for j in range(NL * W):
            wrap[j % 16, j // 16] = vals[j]
        gidx = np.tile(wrap, (8, 1))
        # skip mask on odd lanes: lane 2i+1 may take s-2 if labels differ
        sk = np.zeros((NL, S), dtype=np.float32)
        for n in range(NL):
            for i in range(1, S):
                if lab[n, i] != lab[n, i - 1]:
                    sk[n, i] = 1.0
        em = np.zeros((NL, W), dtype=np.float32)
        for n in range(NL):
            em[n, 2 * lln[n]] = 1.0
            em[n, 2 * lln[n] - 1] = 1.0
        ninvl = (-1.0 / np.maximum(lln, 1)).astype(np.float32).reshape(NL, 1)
        in_maps.append({
            "preds": np.ascontiguousarray(preds[:, ns, :], dtype=np.float32),
            "gidx": gidx,
            "skodd": sk,
            "em01": em,
            "ninvl": ninvl,
        })
    return in_maps


def run_device(preds, labels, label_lengths, T=T_FULL, trace=False):
    if T not in _COMPILED:
        _COMPILED[T] = build_program(T)
    nc = _COMPILED[T]
    in_maps = _host_prep(preds, labels, label_lengths, T)
    res = run_bass_kernel_spmd(nc, in_maps, list(range(NCORES)), trace=trace)
    nlls = np.concatenate([r["nll"].reshape(NL) for r in res.results])
    return nlls, res


def kernel(preds, labels, input_lengths, label_lengths):
    preds = np.asarray(preds)
    labels = np.asarray(labels)
    input_lengths = np.asarray(input_lengths)
    label_lengths = np.asarray(label_lengths)
    assert preds.shape == (T_FULL, N_FULL, C)
    assert int(input_lengths.min()) == T_FULL and int(input_lengths.max()) == T_FULL, \
        "kernel specialized for full-length inputs"
    nlls, _ = run_device(preds, labels, label_lengths)
    # zero_infinity: saturated/non-finite -> 0 (reference semantics)
    nlls = np.where(np.isfinite(nlls) & (np.abs(nlls) < 1e6), nlls, 0.0)
    return np.float32(np.mean(nlls))


# revision 9
# speedup vs baseline: 1.6508x; 1.4037x over previous
"""CTC loss (mean, zero_infinity, target-length normalized) on 8 Trainium2 cores.

Sharding: data-parallel over batch N - each core takes 8 of the 64 sequences,
computes their per-sequence normalized NLL on device; the host mean over the
64 values is the unshard step.

Device algorithm per core (linear/probability space, validated vs f64 on host):
  Phase 1: stream preds (T,8,C); ACT Exp+accumulate produces the per-(t,n)
    softmax denominator; a GPSIMD indirect gather builds a 260-wide
    interleaved lane vector per (t,n): even lanes = blank, odd lanes =
    labels, lanes beyond 2*label_len point at a -1e9 column; ACT Exp with
    per-partition bias (-ln sumexp + lnK) forms p~(t,n,s) = K*softmax prob
    (0 for masked lanes). Streams to HBM as (t, n, 260).
  Phase 2: per-t linear-space CTC forward recursion on (8, 257) state:
    alpha'(s) = (alpha(s) + alpha(s-1) + sk01(s)*alpha(s-2)) * p~(t,s)
    with a scalar renormalization by 1/max every R=8 steps (log accumulated
    on the side); the boost K = e^7 per step keeps the running max centered
    in f32 range. 4 DVE ops per step; renorm adds 2 ops every 8 steps and
    folds its scale into the next step's multiply (scalar_tensor_tensor).
  Final: nll = -(ln(sum of the two end lanes) + sum ln(renorm maxes)
                - T*lnK) / label_len, on device.
"""
import sys
sys.path.insert(0, "/opt/trn_rl_repo")

import numpy as np

import concourse.bass as bass
import concourse.bacc as bacc
import concourse.tile as tile
from concourse import mybir
from concourse.bass_utils import run_bass_kernel_spmd

T_FULL, N_FULL, C, S = 1024, 64, 512, 128
L = 2 * S + 1            # 257 lanes
W = 260                  # stored lane width (257 + 3 pad)
NCORES = 8
NL = N_FULL // NCORES
NEG = -1.0e9
LNK = 7.0                # per-step lattice boost ln K
REN = 8                  # renorm interval
F32 = mybir.dt.float32
U16 = mybir.dt.uint16
AF = mybir.ActivationFunctionType
OP = mybir.AluOpType
TBLK = 32                # lattice t-block size streamed into SBUF

_COMPILED = {}


def build_program(T):
    nc = bacc.Bacc("TRN2", target_bir_lowering=False, debug=False)

    preds = nc.dram_tensor("preds", [T, NL, C], F32, kind="ExternalInput")
    gidx = nc.dram_tensor("gidx", [128, W * NL // 16], U16, kind="ExternalInput")
    skodd = nc.dram_tensor("skodd", [NL, S], F32, kind="ExternalInput")
    em01 = nc.dram_tensor("em01", [NL, W], F32, kind="ExternalInput")
    ninvl = nc.dram_tensor("ninvl", [NL, 1], F32, kind="ExternalInput")
    nll = nc.dram_tensor("nll", [NL, 1], F32, kind="ExternalOutput")

    # p~ lattice in HBM, (n, t, s): DP-side reads are contiguous per seq
    lpd = nc.dram_tensor("lpd", [NL, T, W], F32)

    n_ttiles = T // 128
    n_ren = (T - 2) // REN  # renorms at t = REN-1, 2*REN-1, ..., < T-1

    with tile.TileContext(nc) as tc:
        with (
            tc.tile_pool(name="p1", bufs=2) as p1,
            tc.tile_pool(name="p1s", bufs=2) as p1s,
            tc.tile_pool(name="const", bufs=1) as constp,
            tc.tile_pool(name="dp", bufs=1) as dpp,
            tc.tile_pool(name="lps", bufs=2) as lpsp,
        ):
            # ---------------- constants ----------------
            t_gidx = constp.tile([128, W * NL // 16], U16)
            nc.gpsimd.dma_start(t_gidx[:], gidx[:])
            t_sk = constp.tile([NL, S], F32)
            nc.sync.dma_start(t_sk[:], skodd[:])
            t_em = constp.tile([NL, W], F32)
            nc.sync.dma_start(t_em[:], em01[:])
            t_nil = constp.tile([NL, 1], F32)
            nc.sync.dma_start(t_nil[:], ninvl[:])

            # ---------------- phase 1 ----------------
            for tt in range(n_ttiles):
                ts0 = tt * 128
                tp = p1.tile([128, NL * C + 4], F32, tag="tp")
                nc.gpsimd.dma_start(
                    tp[:, :NL * C],
                    preds[ts0:ts0 + 128].rearrange("t n c -> t (n c)"))
                nc.vector.memset(tp[:, NL * C:], NEG)
                es = p1s.tile([128, C], F32, tag="es")
                sm = p1s.tile([128, NL], F32, tag="sm")
                for n in range(NL):
                    nc.scalar.activation(es[:], tp[:, n * C:(n + 1) * C],
                                         AF.Exp, accum_out=sm[:, n:n + 1])
                lns = p1s.tile([128, NL], F32, tag="lns")
                nc.scalar.activation(lns[:], sm[:], AF.Ln)
                # nb = -ln(sumexp) + lnK
                nb = p1s.tile([128, NL], F32, tag="nb")
                nc.vector.tensor_scalar(nb[:], lns[:], -1.0, LNK,
                                        OP.mult, OP.add)
                g = p1s.tile([128, NL * W], F32, tag="g")
                # ISA caps indirect-copy dst at 1024 elems/partition: chunk it
                for o in range(0, NL * W, 1024):
                    sz = min(1024, NL * W - o)
                    nc.gpsimd.indirect_copy(
                        g[:, o:o + sz], tp[:],
                        t_gidx[:, o // 16:(o + sz) // 16], True)
                lp = p1s.tile([128, NL * W], F32, tag="lp")
                for n in range(NL):
                    nc.scalar.activation(
                        lp[:, n * W:(n + 1) * W], g[:, n * W:(n + 1) * W],
                        AF.Exp, bias=nb[:, n:n + 1], scale=1.0)
                nc.sync.dma_start(
                    lpd[:, ts0:ts0 + 128, :].rearrange("n t k -> t n k"),
                    lp[:].rearrange("t (n k) -> t n k", k=W))

            # ---------------- phase 2: per-t linear-space DP ----------------
            # state: col 0,1 guards (0); col 2+s = alpha(s), s in [0, 257)
            Wd = L + 3
            stA = dpp.tile([NL, Wd], F32)
            stB = dpp.tile([NL, Wd], F32)
            u = dpp.tile([NL, S], F32)
            mxb = dpp.tile([NL, n_ren + 1], F32)
            rcp = dpp.tile([NL, 1], F32)
            nc.vector.memset(stA[:], 0.0)
            nc.vector.memset(stB[:], 0.0)
            state = [stA, stB]

            for blk in range(T // TBLK):
                lpb = lpsp.tile([NL, TBLK * W], F32, tag="lpb")
                nc.sync.dma_start(
                    lpb[:],
                    lpd[:, blk * TBLK:(blk + 1) * TBLK, :].rearrange(
                        "n t k -> n (t k)"))
                for j in range(TBLK):
                    t = blk * TBLK + j
                    X = state[(t + 1) % 2]
                    Y = state[t % 2]
                    pt = lpb[:, j * W:j * W + L]
                    if t == 0:
                        # alpha(0,0) = p(0,blank); alpha(0,1) = p(0,lab0)
                        nc.vector.tensor_copy(Y[:, 2:4], pt[:, 0:2])
                        continue
                    # op1: u = alpha(s-2)|odd * sk01
                    nc.vector.tensor_mul(u[:], X[:, 1:2 + L - 2:2], t_sk[:])
                    # op2: Y = alpha(s) + alpha(s-1)
                    nc.vector.tensor_add(Y[:, 2:2 + L], X[:, 2:2 + L],
                                         X[:, 1:1 + L])
                    # op3: Y|odd += u
                    nc.vector.tensor_add(Y[:, 3:2 + L:2], Y[:, 3:2 + L:2],
                                         u[:])
                    # op4: Y *= p_t  (folding in 1/max after a renorm)
                    if t % REN == 0 and t >= REN:
                        nc.vector.scalar_tensor_tensor(
                            Y[:, 2:2 + L], Y[:, 2:2 + L], rcp[:, 0:1], pt,
                            OP.mult, OP.mult)
                    else:
                        nc.vector.tensor_mul(Y[:, 2:2 + L], Y[:, 2:2 + L], pt)
                    # renorm bookkeeping every REN steps
                    if t % REN == REN - 1 and t < T - 1:
                        jr = t // REN
                        nc.vector.tensor_reduce(
                            mxb[:, jr:jr + 1], Y[:, 2:2 + L],
                            mybir.AxisListType.X, OP.max)
                        nc.vector.reciprocal(rcp[:], mxb[:, jr:jr + 1])

            # ---------------- final ----------------
            Xf = state[(T - 1) % 2]
            tot = dpp.tile([NL, L], F32)
            r = dpp.tile([NL, 1], F32)
            lnm = dpp.tile([NL, n_ren + 1], F32)
            s1 = dpp.tile([NL, 1], F32)
            nc.vector.tensor_mul(tot[:], Xf[:, 2:2 + L], t_em[:, :L])
            nc.vector.tensor_reduce(r[:], tot[:], mybir.AxisListType.X, OP.add)
            nc.scalar.activation(r[:], r[:], AF.Ln)
            nc.scalar.activation(lnm[:, :n_ren], mxb[:, :n_ren], AF.Ln)
            nc.vector.tensor_reduce(s1[:], lnm[:, :n_ren],
                                    mybir.AxisListType.X, OP.add)
            nc.vector.tensor_add(r[:], r[:], s1[:])
            # nll = -(r - T*lnK)/len = (r - T*lnK) * ninvl
            nc.vector.tensor_scalar_add(r[:], r[:], -float(T) * LNK)
            out = dpp.tile([NL, 1], F32)
            nc.vector.tensor_mul(out[:], r[:], t_nil[:])
            nc.sync.dma_start(nll[:], out[:])

    nc.compile()
    return nc


def _host_prep(preds, labels, label_lengths, T):
    labels = np.asarray(labels).astype(np.int64)
    ll = np.asarray(label_lengths).astype(np.int64)
    NEGCOL = NL * C  # column memset to -1e9 in the tp tile
    in_maps = []
    for c in range(NCORES):
        ns = slice(c * NL, (c + 1) * NL)
        lab = labels[ns]
        lln = ll[ns]
        vals = np.zeros(NL * W, dtype=np.uint16)
        for n in range(NL):
            for s in range(W):
                j = n * W + s
                if s >= L:
                    vals[j] = NEGCOL
                elif s % 2 == 0:       # blank lane 2i, alive iff i <= ll
                    vals[j] = (n * C) if (s // 2) <= lln[n] else NEGCOL
                else:                  # label lane 2i+1, alive iff i < ll
                    i = (s - 1) // 2
                    vals[j] = (n * C + lab[n, i]) if i < lln[n] else NEGCOL
        wrap = np.zeros((16, NL * W // 16), dtype=np.uint16)
        for j in range(NL * W):
            wrap[j % 16, j // 16] = vals[j]
        gidx = np.tile(wrap, (8, 1))
        # skip mask on odd lanes: lane 2i+1 may take s-2 if labels differ
        sk = np.zeros((NL, S), dtype=np.float32)
        for n in range(NL):
            for i in range(1, S):
                if lab[n, i] != lab[n, i - 1]:
                    sk[n, i] = 1.0
        em = np.zeros((NL, W), dtype=np.float32)
        for n in range(NL):
            em[n, 2 * lln[n]] = 1.0
            em[n, 2 * lln[n] - 1] = 1.0
        ninvl = (-1.0 / np.maximum(lln, 1)).astype(np.float32).reshape(NL, 1)
        in_maps.append({
            "preds": np.ascontiguousarray(preds[:, ns, :], dtype=np.float32),
            "gidx": gidx,
            "skodd": sk,
            "em01": em,
            "ninvl": ninvl,
        })
    return in_maps


def run_device(preds, labels, label_lengths, T=T_FULL, trace=False):
    if T not in _COMPILED:
        _COMPILED[T] = build_program(T)
    nc = _COMPILED[T]
    in_maps = _host_prep(preds, labels, label_lengths, T)
    res = run_bass_kernel_spmd(nc, in_maps, list(range(NCORES)), trace=trace)
    nlls = np.concatenate([r["nll"].reshape(NL) for r in res.results])
    return nlls, res


def kernel(preds, labels, input_lengths, label_lengths):
    preds = np.asarray(preds)
    labels = np.asarray(labels)
    input_lengths = np.asarray(input_lengths)
    label_lengths = np.asarray(label_lengths)
    assert preds.shape == (T_FULL, N_FULL, C)
    assert int(input_lengths.min()) == T_FULL and int(input_lengths.max()) == T_FULL, \
        "kernel specialized for full-length inputs"
    nlls, _ = run_device(preds, labels, label_lengths)
    # zero_infinity: saturated/non-finite -> 0 (reference semantics)
    nlls = np.where(np.isfinite(nlls) & (np.abs(nlls) < 1e6), nlls, 0.0)
    return np.float32(np.mean(nlls))
